# revision 14
# baseline (speedup 1.0000x reference)
"""Multi-head attention (B=4, S=2048, E=768, H=8, D=96) on 8 TRN2 NeuronCores.

Sharding: token-split — core c handles batch b=c//2, query-half qh=c%2
(1024 queries), computing K/V over the batch's full 2048 keys (redundantly
across the 2 cores of a batch pair). No collectives.

Device-side dataflow per core (all matmuls in fp32r — full-rate PE mode,
~1.8e-4 relative error):
  - x^T [768, 2048] in SBUF, key order rotated so this core's queries are
    always columns 0:1024 (softmax over keys is order-invariant).
  - Q^T_h [96,1024], K^T_h [96,2048] via projection matmuls
    (lhsT = W columns, rhs = x^T).
  - V for a group of 4 heads at a time, in [token, head-dim] layout with an
    extra all-ones column per head (from an appended ones-row of x^T), so the
    PV matmul also produces the softmax denominator for free.
  - energy^T tiles [128 keys, 1024 queries] = K^T_h-tile.T @ Q^T_h; exp on
    ACT (no max subtraction: |energy| <~ 25, safe in fp32); PV accumulates
    out^T [97, 1024] over 16 key tiles.
  - normalization: row 96 of out^T = sum of exp; recip * 96, partition-
    broadcast (Pool engine), multiply in place, spill to DRAM.
  - final projection: out[tok, :] = sum_h attnout^T_h.T @ Wp[head rows],
    accumulated in PSUM over heads, DMA'd straight to the output.

Biases: bq/bk/bv fold in via an extra x^T ones-row matmul step (only
compiled in when nonzero); bp is added on the host.
"""

import os
import sys

import numpy as np

try:
    import concourse.bass as bass  # noqa: F401
except ImportError:  # pragma: no cover - fallback for odd sys.path setups
    for p in (
        "/root/.axon_site",
        "/root/.axon_site/_ro/trn_rl_repo",
        "/root/.axon_site/_ro/pypackages",
        "/opt/trn_rl_repo",
    ):
        if os.path.isdir(p) and p not in sys.path:
            sys.path.append(p)
    import concourse.bass as bass  # noqa: F401

import concourse.mybir as mybir
from concourse import bacc
from concourse.bass_utils import run_bass_kernel_spmd
from concourse.tile import TileContext

B, S, E, H, D = 4, 2048, 768, 8, 96
NQ = S // 2          # queries per core
NCORES = 8
KT_N = S // 128      # 16 key tiles
VCH = 4 * (D + 1)    # V-group chunk width: 4 heads x (96 dims + ones col)
F32 = mybir.dt.float32
F32R = mybir.dt.float32r
EXP = mybir.ActivationFunctionType.Exp

_CACHE: dict = {}

# Filled by the last kernel() call (for test harnesses): exec_time_ns etc.
LAST_RESULT = {}


def _build(has_bias: bool):
    nc = bacc.Bacc("TRN2", target_bir_lowering=False, debug=False,
                   num_devices=NCORES)
    xT_d = nc.dram_tensor("xT", [E + 1, S], F32R, kind="ExternalInput").ap()
    wq_d = nc.dram_tensor("Wq", [E + 1, E], F32R, kind="ExternalInput").ap()
    wk_d = nc.dram_tensor("Wk", [E + 1, E], F32R, kind="ExternalInput").ap()
    wv_d = nc.dram_tensor("Wv", [E + 1, 2 * VCH], F32R, kind="ExternalInput").ap()
    wp_d = nc.dram_tensor("Wp", [H, D, E], F32R, kind="ExternalInput").ap()
    out_d = nc.dram_tensor("out", [NQ, E], F32, kind="ExternalOutput").ap()

    with TileContext(nc) as tc:
        with (
            tc.tile_pool(name="dr", bufs=1, space="DRAM") as drpool,
            tc.tile_pool(name="dn", bufs=1) as dnpool,
        ):
            attn_dram = drpool.tile([D, H * NQ], F32)
            den_sb = [dnpool.tile([97, NQ], F32, tag="denA", name="denA"),
                      dnpool.tile([97, NQ], F32, tag="denB", name="denB")]
            nc.gpsimd.memset(den_sb[0][:], 1.0)
            nc.gpsimd.memset(den_sb[1][:], 1.0)

            with (
                tc.tile_pool(name="w", bufs=1) as wpool,
                tc.tile_pool(name="qt", bufs=2) as qtpool,
                tc.tile_pool(name="kt", bufs=2) as ktpool,
                tc.tile_pool(name="vg", bufs=1) as vgpool,
                tc.tile_pool(name="pt", bufs=3) as ptpool,
                tc.tile_pool(name="mi", bufs=2) as mipool,
                tc.tile_pool(name="mm", bufs=4, space="PSUM") as mmps,
                tc.tile_pool(name="pv", bufs=4, space="PSUM") as pvps,
            ):
                # --- resident loads -------------------------------------
                xt = [wpool.tile([128, S], F32R, tag=f"xt{k}", name=f"xt{k}")
                      for k in range(6)]
                # query-half columns first: unblocks Q^T/K^T production ASAP
                for k in range(6):
                    nc.sync.dma_start(xt[k][:, 0:NQ],
                                      xT_d[128 * k:128 * (k + 1), 0:NQ])
                wq, wk, wv = [], [], []
                for nm, dram, lst, width in (
                    ("wq", wq_d, wq, E),
                    ("wk", wk_d, wk, E),
                    ("wv", wv_d, wv, 2 * VCH),
                ):
                    for k in range(6):
                        t = wpool.tile([128, width], F32R, tag=f"{nm}{k}")
                        nc.sync.dma_start(t[:], dram[128 * k:128 * (k + 1), :])
                        lst.append(t)
                    if nm == "wk":
                        for k in range(6):
                            nc.sync.dma_start(
                                xt[k][:, NQ:S],
                                xT_d[128 * k:128 * (k + 1), NQ:S])
                ones = wpool.tile([1, S], F32R, tag="ones")
                nc.sync.dma_start(ones[:], xT_d[E:E + 1, :])
                wvb = wpool.tile([1, 2 * VCH], F32R, tag="wvb")
                nc.sync.dma_start(wvb[:], wv_d[E:E + 1, :])
                if has_bias:
                    wqb = wpool.tile([1, E], F32R, tag="wqb")
                    nc.sync.dma_start(wqb[:], wq_d[E:E + 1, :])
                    wkb = wpool.tile([1, E], F32R, tag="wkb")
                    nc.sync.dma_start(wkb[:], wk_d[E:E + 1, :])

                # (weight tile, x^T tile) pairs per contraction step
                q_steps = [(wq[k], xt[k]) for k in range(6)]
                k_steps = [(wk[k], xt[k]) for k in range(6)]
                v_steps = [(wv[k], xt[k]) for k in range(6)] + [(wvb, ones)]
                if has_bias:
                    q_steps.append((wqb, ones))
                    k_steps.append((wkb, ones))

                vg = None
                for h in range(H):
                    g, j = divmod(h, 4)
                    c0, c1 = D * h, D * h + D

                    # --- Q^T_h [96, 1024] ------------------------------
                    QT = qtpool.tile([D, NQ], F32R, tag="qt")
                    qps = [mmps.tile([D, 512], F32, tag="mm", name=f"q{h}{qc}")
                           for qc in range(2)]
                    for s, (wt, xs) in enumerate(q_steps):
                        for qc in range(2):
                            nc.tensor.matmul(
                                qps[qc][:], (wt[:, c0:c1]),
                                (xs[:, 512 * qc:512 * qc + 512]),
                                start=(s == 0), stop=(s == len(q_steps) - 1))
                    for qc in range(2):
                        nc.vector.tensor_copy(QT[:, 512 * qc:512 * qc + 512],
                                              qps[qc][:])

                    # --- K^T_h [96, 2048] ------------------------------
                    KT = ktpool.tile([D, S], F32R, tag="kt")
                    for half in range(2):
                        kps = [mmps.tile([D, 512], F32, tag="mm",
                                         name=f"k{h}{half}{i}")
                               for i in range(2)]
                        for s, (wt, xs) in enumerate(k_steps):
                            for i in range(2):
                                kc = 2 * half + i
                                nc.tensor.matmul(
                                    kps[i][:], (wt[:, c0:c1]),
                                    (xs[:, 512 * kc:512 * kc + 512]),
                                    start=(s == 0),
                                    stop=(s == len(k_steps) - 1))
                        for i in range(2):
                            kc = 2 * half + i
                            nc.vector.tensor_copy(
                                KT[:, 512 * kc:512 * kc + 512], kps[i][:])

                    # --- V for heads 4g..4g+3, [tok, 4*(96+1)] ---------
                    if j == 0:
                        vg = vgpool.tile([128, KT_N * VCH], F32R, tag="vg")
                        for t in range(KT_N):
                            ps = mmps.tile([128, VCH], F32, tag="mm")
                            for s, (wt, xs) in enumerate(v_steps):
                                nc.tensor.matmul(
                                    ps[:], (xs[:, 128 * t:128 * (t + 1)]),
                                    (wt[:, VCH * g:VCH * (g + 1)]),
                                    start=(s == 0), stop=(s == len(v_steps) - 1))
                            nc.vector.tensor_copy(
                                vg[:, VCH * t:VCH * t + VCH], ps[:])

                    # --- attention for head h --------------------------
                    pvc = [pvps.tile([D + 1, 512], F32, tag="pv",
                                     name=f"pv{h}_{qc}")
                           for qc in range(2)]
                    voff = 97 * j
                    for i in range(KT_N):
                        pT = ptpool.tile([128, NQ], F32R, tag="pt")
                        for qc in range(2):
                            ps = mmps.tile([128, 512], F32, tag="mm")
                            nc.tensor.matmul(
                                ps[:], (KT[:, 128 * i:128 * (i + 1)]),
                                (QT[:, 512 * qc:512 * qc + 512]),
                                start=True, stop=True)
                            nc.scalar.activation(
                                pT[:, 512 * qc:512 * qc + 512], ps[:], EXP)
                        for qc in range(2):
                            nc.tensor.matmul(
                                pvc[qc][:],
                                (vg[:, VCH * i + voff:VCH * i + voff + D + 1]),
                                (pT[:, 512 * qc:512 * qc + 512]),
                                start=(i == 0), stop=(i == KT_N - 1))

                    # --- normalize (x96 / rowsum) and spill ------------
                    stg = mipool.tile([D, NQ], F32, tag="stg")
                    dt_, dp = den_sb[h // 4], 32 * (h % 4)
                    for qc in range(2):
                        nc.vector.tensor_copy(
                            dt_[dp:dp + 1, 512 * qc:512 * qc + 512],
                            pvc[qc][D:D + 1, :])
                        nc.vector.tensor_copy(stg[:, 512 * qc:512 * qc + 512],
                                              pvc[qc][0:D, :])
                    nc.sync.dma_start(attn_dram[:, NQ * h:NQ * (h + 1)],
                                      stg[:])

            # --- final projection: out = attnout @ Wp (+bp on host) ----
            with (
                tc.tile_pool(name="fw", bufs=1) as fwpool,
                tc.tile_pool(name="fs", bufs=3) as fspool,
                tc.tile_pool(name="fm", bufs=4, space="PSUM") as fmps,
            ):
                rcp = [dnpool.tile([97, NQ], F32, tag="rcpA", name="rcpA"),
                       dnpool.tile([97, NQ], F32, tag="rcpB", name="rcpB")]
                nc.vector.reciprocal(rcp[0][:], den_sb[0][:])
                nc.vector.reciprocal(rcp[1][:], den_sb[1][:])
                wp_t, at_n = [], []
                for h in range(H):
                    wt = fwpool.tile([D, E], F32R, tag=f"wp{h}")
                    nc.sync.dma_start(wt[:], wp_d[h])
                    wp_t.append(wt)
                    at = fspool.tile([D, NQ], F32, tag="at", name=f"at{h}")
                    nc.sync.dma_start(at[:], attn_dram[:, NQ * h:NQ * (h + 1)])
                    rt, rp = rcp[h // 4], 32 * (h % 4)
                    tmp = fspool.tile([1, NQ], F32, tag="rtmp",
                                      name=f"rtmp{h}")
                    nc.vector.tensor_copy(tmp[:], rt[rp:rp + 1, :])
                    bc = fspool.tile([D, NQ], F32, tag="bc", name=f"bc{h}")
                    nc.gpsimd.partition_broadcast(bc[:], tmp[:])
                    an = fwpool.tile([D, NQ], F32R, tag=f"an{h}")
                    nc.vector.tensor_mul(an[:], at[:], bc[:])
                    at_n.append(an)
                CHUNKS = ((0, 512), (512, 256))
                for t in range(NQ // 128):
                    fps = [fmps.tile([128, cw], F32, tag="fm",
                                     name=f"f{t}{ci}")
                           for ci, (cs, cw) in enumerate(CHUNKS)]
                    for h in range(H):
                        for ci, (cs, cw) in enumerate(CHUNKS):
                            nc.tensor.matmul(
                                fps[ci][:], (at_n[h][:, 128 * t:128 * (t + 1)]),
                                (wp_t[h][:, cs:cs + cw]),
                                start=(h == 0), stop=(h == H - 1))
                    for ci, (cs, cw) in enumerate(CHUNKS):
                        fo = fspool.tile([128, cw], F32, tag="fo",
                                         name=f"fo{t}{ci}")
                        nc.scalar.copy(fo[:], fps[ci][:])
                        nc.sync.dma_start(
                            out_d[128 * t:128 * (t + 1), cs:cs + cw], fo[:])

    nc.compile()
    return nc


def _prep_inputs(x, Wq, bq, Wk, bk, Wv, bv, Wp):
    """Host-side shard prep. Returns (has_bias, per-core in_maps)."""
    has_bias = bool(np.any(bq) or np.any(bk) or np.any(bv))
    wq_aug = np.ascontiguousarray(np.vstack([Wq, bq[None, :]]), dtype=np.float32)
    wk_aug = np.ascontiguousarray(np.vstack([Wk, bk[None, :]]), dtype=np.float32)
    wv_grp = np.zeros((E + 1, 2 * VCH), dtype=np.float32)
    for h in range(H):
        g, j = divmod(h, 4)
        base = VCH * g + 97 * j
        wv_grp[:E, base:base + D] = Wv[:, D * h:D * h + D]
        wv_grp[E, base:base + D] = bv[D * h:D * h + D]
        wv_grp[E, base + D] = 1.0  # ones column (selects x ones-row)
    wp_r = np.ascontiguousarray(Wp.reshape(H, D, E) * float(D),
                            dtype=np.float32)

    in_maps = []
    for c in range(NCORES):
        b, qh = divmod(c, 2)
        xb = x[b]
        if qh == 0:
            xc = xb
        else:
            xc = np.concatenate([xb[NQ:], xb[:NQ]], axis=0)
        xT = np.empty((E + 1, S), dtype=np.float32)
        xT[:E] = xc.T
        xT[E] = 1.0
        in_maps.append({"xT": xT, "Wq": wq_aug, "Wk": wk_aug,
                       "Wv": wv_grp, "Wp": wp_r})
    return has_bias, in_maps


def kernel(x, Wq, bq, Wk, bk, Wv, bv, Wp, bp):
    x = np.asarray(x, dtype=np.float32)
    Wq = np.asarray(Wq, dtype=np.float32)
    bq = np.asarray(bq, dtype=np.float32)
    Wk = np.asarray(Wk, dtype=np.float32)
    bk = np.asarray(bk, dtype=np.float32)
    Wv = np.asarray(Wv, dtype=np.float32)
    bv = np.asarray(bv, dtype=np.float32)
    Wp = np.asarray(Wp, dtype=np.float32)
    bp = np.asarray(bp, dtype=np.float32)
    assert x.shape == (B, S, E), x.shape

    has_bias, in_maps = _prep_inputs(x, Wq, bq, Wk, bk, Wv, bv, Wp)

    if has_bias not in _CACHE:
        _CACHE[has_bias] = _build(has_bias)
    nc = _CACHE[has_bias]

    trace = bool(os.environ.get("BASS_TRACE"))
    if trace and "antenv.axon_hooks" not in sys.modules:
        _register_ntff_shim()
    res = run_bass_kernel_spmd(nc, in_maps, list(range(NCORES)), trace=trace)

    LAST_RESULT.clear()
    LAST_RESULT.update(
        exec_time_ns=res.exec_time_ns,
        mean_exec_time_ns=res.mean_exec_time_ns,
        instructions_and_trace=res.instructions_and_trace,
        profile_json=res.profile_json,
    )

    out = np.empty((B, S, E), dtype=np.float32)
    for c in range(NCORES):
        b, qh = divmod(c, 2)
        out[b, qh * NQ:(qh + 1) * NQ] = res.results[c]["out"]
    if np.any(bp):
        out += bp[None, None, :]
    return out


def _register_ntff_shim():
    """Make run_bass_kernel_spmd's NTFF profiling work in containers that
    lack antenv.axon_hooks (profiles via ctypes into libaxon_pjrt.so)."""
    import contextlib
    import ctypes
    import types

    so = "/opt/axon/libaxon_pjrt.so"
    if not os.path.exists(so):
        return
    lib = ctypes.CDLL(so)
    if not hasattr(lib, "axon_start_nrt_profile"):
        return
    lib.axon_start_nrt_profile.argtypes = [ctypes.POINTER(ctypes.c_int64),
                                           ctypes.c_size_t]
    lib.axon_start_nrt_profile.restype = ctypes.c_int64
    lib.axon_stop_nrt_profile.argtypes = [ctypes.c_char_p]
    lib.axon_stop_nrt_profile.restype = ctypes.c_int64

    @contextlib.contextmanager
    def _hook(output_dir, device_ids):
        import jax

        jax.devices()
        if device_ids:
            ids = (ctypes.c_int64 * len(device_ids))(*device_ids)
            rc = lib.axon_start_nrt_profile(ids, len(device_ids))
        else:
            rc = lib.axon_start_nrt_profile(None, 0)
        if rc != 0:
            raise RuntimeError(f"axon_start_nrt_profile rc={rc}")
        try:
            yield
        finally:
            n = lib.axon_stop_nrt_profile(str(output_dir).encode())
            print(f"ntff profile: {n} file(s) -> {output_dir}", file=sys.stderr)

    mod = types.ModuleType("antenv.axon_hooks")
    mod.get_axon_ntff_profile_hook = lambda: _hook
    mod.set_axon_ntff_profile_hook = lambda h: None
    sys.modules["antenv.axon_hooks"] = mod
